# revision 25
# baseline (speedup 1.0000x reference)
"""Trainium2 Bass kernel for nn_CubicModelLarge (3-layer cubic-feature MLP).

Tensor-parallel over the cubic min-index p (64 values, 8 per core; core c,
slot s -> p = 8s + c).  Monomial folding: each cubic monomial x_p x_q x_r
(p<=q<=r) is accumulated once into block p, contracting the triu pair basis

  u_(q,r) = (x_q + x_r)^2 / 2   (q < r, q-major tail order)
  sq_q    = x_q^2
  x_k     = x_k                 (carries the folded quadratic weights)

Block p only needs rows with q >= p, so chunk-level tail skipping cuts the
streamed GEMM columns ~2x vs the unfolded basis (slot-prefix packing keeps
the schedule SPMD-uniform; shorter-tail cores just carry zero weights).

Per core, per layer:
  H[b,(s,o)] = sum_rows F[row,b] * Wfold[row,(s,o)]    (fp16 GEMM, fp32 PSUM)
  y_c[b,o]   = lin[b,o] + sum_s xmac[b,s] * H[b,(s,o)]  (DVE mult+reduce)
  y          = AllReduce_c(y_c)

The batch is processed in thirds (384/384/256) with one fp16 AllReduce per
third, so each collective's latency hides behind the other thirds' compute.
u rows are built by a selection-SUM matmul on the PE (two 1s per column)
followed by a Square activation on the Scalar engine (PSUM->SBUF fp16).
A tiny warm-up AllReduce issues first to absorb collective-init latency.
Final layer partials are summed on the host.
"""

import numpy as np

D = 64
B = 1024
NCORES = 8
NSLOT = D // NCORES          # 8 slots (i-values) per core
OUTS = (64, 64, 10)
NUC = 16                     # u chunks (2016 rows + pad)
NCHUNK = 17                  # + 1 [sq; x] chunk
INV_SQRT2 = 0.7071067811865476

THIRDS = [(0, 384), (384, 768), (768, 1024)]
NT = len(THIRDS)

# slot s covers p in [8s, 8s+8); its tail starts at u-row off(8s)
_OFF = [q * (127 - q) // 2 for q in range(D)]
SLOT_START = [_OFF[8 * s] // 128 for s in range(NSLOT)]      # [0,3,6,9,11,13,14,15]
NSLOTS_AT = [sum(1 for s in range(NSLOT) if SLOT_START[s] <= c) for c in range(NUC)] + [NSLOT]
CHUNK_ORDER = [15, 16, 14, 13, 12, 11, 10, 9, 8, 7, 6, 5, 4, 3, 2, 1, 0]

_CACHE = {}


# ---------------------------------------------------------------- host prep --

def _pair_rows():
    """u-row index map: rows 0..2015 are pairs (q<r) q-major."""
    Q = np.zeros(2016, np.int64)
    R = np.zeros(2016, np.int64)
    for q in range(D):
        o = _OFF[q]
        n = 63 - q
        Q[o:o + n] = q
        R[o:o + n] = np.arange(q + 1, D)
    return Q, R


def _fold_blocks(W, out):
    """-> G [17*128, 64(p), out] folded coefficients per block p."""
    W_sq = W[:, D:D + 2080]
    W_cu = W[:, D + 2080:].reshape(out, D, 2080)
    iu, ju = np.triu_indices(D)

    # T3[p, q, r, out]: sum of W_cu[o, i, (j,k)] over placements, sorted triple
    T3 = np.zeros((D, D, D, out), np.float32)
    I = np.repeat(np.arange(D), 2080)
    J = np.tile(iu, D)
    K = np.tile(ju, D)
    S = np.sort(np.stack([I, J, K]), axis=0)
    V = W_cu.transpose(1, 2, 0).reshape(-1, out)
    np.add.at(T3, (S[0], S[1], S[2]), V)

    Q, R = _pair_rows()
    G = np.zeros((NCHUNK * 128, D, out), np.float32)
    # u-rows: G[row(q,r), p] = T3[p, q, r]  (zero when q < p by construction)
    G[:2016] = T3[:, Q, R, :].transpose(1, 0, 2)
    # sq-rows: diag cubic minus u-substitution corrections
    rowsum = T3.sum(axis=2)                     # [p, q, out] : sum_r T3[p,q,r]
    colsum = T3.sum(axis=1)                     # [p, r, out] : sum_q T3[p,q,r]
    diag = T3[:, np.arange(D), np.arange(D), :]  # [p, q, out]
    sqco = diag - 0.5 * (rowsum + colsum - 2 * diag)
    G[2048:2048 + D] = sqco.transpose(1, 0, 2)
    # x-rows: folded quadratic, pairs with min = p
    tmap = np.zeros((D, D), np.int64)
    tmap[iu, ju] = np.arange(2080)
    tmap[ju, iu] = tmap[iu, ju]
    Wsym = W_sq[:, tmap]                        # [out, p, k]
    mask = (np.arange(D)[None, :] >= np.arange(D)[:, None]).astype(np.float32)
    G[2112:2112 + D] = (Wsym * mask[None]).transpose(2, 1, 0)
    return G


def _prep_layer(W, b, out):
    """-> (wcub [NCORES](17*128, NSLOT*out) fp16, wlin [NCORES](65, out) fp16)"""
    G = _fold_blocks(W, out)
    wcubs, wlins = [], []
    for core in range(NCORES):
        wcub = np.zeros((NCHUNK * 128, NSLOT * out), np.float32)
        for s in range(NSLOT):
            wcub[:, s * out:(s + 1) * out] = G[:, 8 * s + core, :]
        wcubs.append(np.ascontiguousarray(wcub.astype(np.float16)))
        wl = np.zeros((65, out), np.float32)
        if core == 0:
            wl[:D] = W[:, :D].T
            wl[D] = b
        wlins.append(wl.astype(np.float16))
    return wcubs, wlins


def _sel_consts():
    """Selection-SUM matrices (64, 17*128), fp16.

    chunk c<16, col p: +1 at rows Q[128c+p], R[128c+p] (zero cols past 2016).
    chunk 16: col a (a<64): +1 at row a (builds x_a, squared to x_a^2).
    """
    Q, R = _pair_rows()
    sel = np.zeros((D, NCHUNK * 128), np.float16)
    for rho in range(2016):
        sel[Q[rho], rho] += 1.0
        sel[R[rho], rho] += 1.0
    for a in range(D):
        sel[a, NUC * 128 + a] += 1.0
    return sel


# ------------------------------------------------------------------ builder --

def _build_module():
    import concourse.bacc as bacc
    import concourse.mybir as mybir
    import concourse.tile as tile

    F32 = mybir.dt.float32
    F16 = mybir.dt.float16
    MULT = mybir.AluOpType.mult
    ADD = mybir.AluOpType.add
    SQUARE = mybir.ActivationFunctionType.Square
    AXIS_X = mybir.AxisListType.X

    nc = bacc.Bacc("TRN2", target_bir_lowering=False, num_devices=NCORES, debug=False)

    x_in = nc.dram_tensor("x", [B, D], F32, kind="ExternalInput")
    wcub_in = [
        nc.dram_tensor(f"wcub{li}", [NCHUNK * 128, NSLOT * OUTS[li]], F16, kind="ExternalInput")
        for li in range(3)
    ]
    wlin_in = [
        nc.dram_tensor(f"wlin{li}", [65, OUTS[li]], F16, kind="ExternalInput")
        for li in range(3)
    ]
    colsel_in = nc.dram_tensor("colsel", [D, NSLOT], F16, kind="ExternalInput")
    out_ext = nc.dram_tensor("out", [B, OUTS[2]], F32, kind="ExternalOutput")

    sel_c = nc.inline_tensor(_sel_consts(), name="selc")
    ident_c = nc.inline_tensor(np.eye(128, dtype=np.float32), name="identc")
    ident16_c = nc.inline_tensor(np.eye(128, dtype=np.float16), name="ident16c")

    with tile.TileContext(nc) as tc:
        with (
            tc.tile_pool(name="wpool", bufs=2) as wpool,
            tc.tile_pool(name="spool", bufs=1) as spool,
            tc.tile_pool(name="xpool", bufs=2) as xpool,
            tc.tile_pool(name="qpool", bufs=1) as qpool,
            tc.tile_pool(name="ypool", bufs=2) as ypool,
            tc.tile_pool(name="hpool", bufs=3) as hpool,
            tc.tile_pool(name="ps_rep", bufs=2, space="PSUM") as ps_rep,
            tc.tile_pool(name="ps_h", bufs=3, space="PSUM") as ps_h,
            tc.tile_pool(name="ps_small", bufs=3, space="PSUM") as ps_small,
            tc.tile_pool(name="dpool", bufs=2, space="DRAM") as dpool,
        ):
            # ---- warm-up collective: absorb ncfw init + cross-core skew
            warm_src = dpool.tile([128, 4], F16, tag="warm_src")
            warm_dst = dpool.tile([128, 4], F16, tag="warm_dst")
            warm_sb = spool.tile([128, 4], F16, tag="warm_sb")
            nc.vector.memset(warm_sb[:], 0.0)
            nc.sync.dma_start(warm_src[:], warm_sb[:])
            nc.gpsimd.collective_compute(
                "AllReduce",
                ADD,
                replica_groups=[list(range(NCORES))],
                ins=[warm_src.opt()],
                outs=[warm_dst.opt()],
            )

            sel_sb = spool.tile([D, NCHUNK * 128], F16, tag="sel")
            nc.scalar.dma_start(sel_sb[:], sel_c.ap())
            ident_sb = spool.tile([128, 128], F32, tag="ident")
            nc.scalar.dma_start(ident_sb[:], ident_c.ap())
            ident16_sb = spool.tile([128, 128], F16, tag="ident16")
            nc.scalar.dma_start(ident16_sb[:], ident16_c.ap())
            colsel_sb = spool.tile([D, NSLOT], F16, tag="colsel")
            nc.scalar.dma_start(colsel_sb[:], colsel_in.ap())

            # per-layer weight tiles; only the active slot-prefix per chunk.
            # weight DMAs ride the vector queue so they never head-of-line
            # block the latency-critical bounce/x DMAs on the sync queue.
            weights = []
            for li in range(3):
                out_l = OUTS[li]
                M = NSLOT * out_l
                wcub_sb = wpool.tile([128, NCHUNK, M], F16, tag="wcub")
                for c in range(NCHUNK):
                    w = out_l * NSLOTS_AT[c]
                    nc.scalar.dma_start(
                        wcub_sb[:, c, 0:w],
                        wcub_in[li].ap()[c * 128:(c + 1) * 128, 0:w],
                    )
                wlin_sb = wpool.tile([65, out_l], F16, tag="wlin")
                nc.scalar.dma_start(wlin_sb[:], wlin_in[li].ap())
                weights.append((wcub_sb, wlin_sb))

            # x tiles for layer 0 (fp32 straight from the input)
            x_parts = []
            for t, (b0, b1) in enumerate(THIRDS):
                nbt = (b1 - b0) // 128
                xs = xpool.tile([128, nbt, D], F32, tag=f"x{t}")
                nc.sync.dma_start(
                    xs[:],
                    x_in.ap()[b0:b1, :].rearrange("(bc p) f -> p bc f", p=128),
                )
                x_parts.append(xs)

            for li in range(3):
                out_l = OUTS[li]
                M = NSLOT * out_l
                last = li == 2
                wcub_sb, wlin_sb = weights[li]
                next_x = [None] * NT

                for t, (b0, b1) in enumerate(THIRDS):
                    TB = b1 - b0
                    nbt = TB // 128
                    x_sb = x_parts[t]
                    idw = ident_sb if li == 0 else ident16_sb

                    # -- phase A: xT via PE transposes + cast
                    xT_sb = xpool.tile([65, TB], F16, tag=f"xT{t}")
                    for bc in range(nbt):
                        xTp = ps_small.tile([D, 128], F32 if li == 0 else F16, tag="small")
                        nc.tensor.transpose(xTp[:], x_sb[:, bc, :], idw[:])
                        nc.scalar.copy(xT_sb[0:D, bc * 128:(bc + 1) * 128], xTp[:])
                    nc.vector.memset(xT_sb[D:65, :], 1.0)

                    # -- phase B: u chunks (sel-sum matmul + Square); chunk 16 = [sq; x]
                    xsq = []
                    for c in range(NUC):
                        rep = ps_rep.tile([128, TB], F32, tag="rep")
                        nc.tensor.matmul(
                            rep[:], sel_sb[:, c * 128:(c + 1) * 128],
                            xT_sb[0:D, :], start=True, stop=True,
                        )
                        xq = qpool.tile([128, TB], F16, tag=f"xsq{c}t{t}")
                        nc.scalar.activation(xq[:], rep[:], SQUARE, scale=INV_SQRT2)
                        xsq.append(xq)
                    rep16 = ps_rep.tile([128, TB], F32, tag="rep")
                    nc.tensor.matmul(
                        rep16[0:D, :], sel_sb[:, NUC * 128:NUC * 128 + D],
                        xT_sb[0:D, :], start=True, stop=True,
                    )
                    xq16 = qpool.tile([128, TB], F16, tag=f"xsq16t{t}")
                    nc.scalar.activation(xq16[0:D, :], rep16[0:D, :], SQUARE, scale=1.0)
                    nc.vector.tensor_copy(xq16[D:128, :], xT_sb[0:D, :])
                    xsq.append(xq16)

                    # -- phase C
                    y_sb = ypool.tile([128, nbt, out_l], F16, tag=f"y{t}")
                    for bc in range(nbt):
                        bs = slice(bc * 128, (bc + 1) * 128)
                        h_ps = ps_h.tile([128, M], F32, tag="h")
                        for j, c in enumerate(CHUNK_ORDER):
                            w = out_l * NSLOTS_AT[c]
                            nc.tensor.matmul(
                                h_ps[:, 0:w], xsq[c][:, bs], wcub_sb[:, c, 0:w],
                                start=(j == 0), stop=(j == NCHUNK - 1),
                            )

                        lin_ps = ps_small.tile([128, out_l], F32, tag="small")
                        nc.tensor.matmul(lin_ps[:], xT_sb[0:65, bs], wlin_sb[:], start=True, stop=True)
                        xmac_ps = ps_small.tile([128, NSLOT], F32, tag="small")
                        nc.tensor.matmul(xmac_ps[:], xT_sb[0:D, bs], colsel_sb[:], start=True, stop=True)
                        xmac_sb = ypool.tile([128, NSLOT], F32, tag="xmac")
                        nc.scalar.copy(xmac_sb[:], xmac_ps[:])

                        # tmp[:, :M] = h * xmac (broadcast over o); tmp[:, M:] = lin
                        tmp_sb = hpool.tile([128, M + out_l], F32, tag="tmp")
                        xmac_b = (
                            xmac_sb[:].unsqueeze(2).to_broadcast([128, NSLOT, out_l])
                        )
                        nc.vector.tensor_tensor(
                            tmp_sb[:, 0:M].rearrange("p (i o) -> p i o", i=NSLOT),
                            h_ps[:].rearrange("p (i o) -> p i o", i=NSLOT),
                            xmac_b,
                            op=MULT,
                        )
                        nc.scalar.copy(tmp_sb[:, M:], lin_ps[:])
                        with nc.allow_low_precision(reason="y partials are fp16 for the collective"):
                            nc.vector.tensor_reduce(
                                y_sb[:, bc, :],
                                tmp_sb[:].rearrange("p (i o) -> p o i", i=NSLOT + 1),
                                axis=AXIS_X, op=ADD,
                            )

                    if not last:
                        # -- phase D: AllReduce this third (fp16 payload)
                        y_bounce = dpool.tile([TB, out_l], F16, tag=f"ybounce{t}")
                        y_red = dpool.tile([TB, out_l], F16, tag=f"yred{t}")
                        nc.sync.dma_start(
                            y_bounce[:].rearrange("(bc p) o -> p bc o", p=128), y_sb[:]
                        )
                        nc.gpsimd.collective_compute(
                            "AllReduce",
                            ADD,
                            replica_groups=[list(range(NCORES))],
                            ins=[y_bounce.opt()],
                            outs=[y_red.opt()],
                        )
                        xs = xpool.tile([128, nbt, D], F16, tag=f"xn{t}")
                        nc.sync.dma_start(
                            xs[:], y_red[:].rearrange("(bc p) f -> p bc f", p=128)
                        )
                        next_x[t] = xs
                    else:
                        y32_sb = ypool.tile([128, nbt, out_l], F32, tag=f"y32{t}")
                        nc.scalar.copy(y32_sb[:], y_sb[:])
                        nc.sync.dma_start(
                            out_ext.ap()[b0:b1, :]
                            .rearrange("(bc p) o -> p bc o", p=128),
                            y32_sb[:],
                        )

                if not last:
                    x_parts = next_x

    nc.compile()
    return nc


# ------------------------------------------------------------------- runner --

def kernel(x, W0, b0, W1, b1, W2, b2):
    from concourse.bass_utils import run_bass_kernel_spmd

    if "nc" not in _CACHE:
        _CACHE["nc"] = _build_module()
    nc = _CACHE["nc"]

    x = np.ascontiguousarray(np.asarray(x, np.float32))
    Ws = [np.asarray(W, np.float32) for W in (W0, W1, W2)]
    bs = [np.asarray(b_, np.float32) for b_ in (b0, b1, b2)]

    wcubs, wlins = {}, {}
    for li in range(3):
        wcubs[li], wlins[li] = _prep_layer(Ws[li], bs[li], OUTS[li])

    in_maps = []
    for core in range(NCORES):
        colsel = np.zeros((D, NSLOT), np.float16)
        for s in range(NSLOT):
            colsel[8 * s + core, s] = 1.0
        m = {"x": x, "colsel": colsel}
        for li in range(3):
            m[f"wcub{li}"] = wcubs[li][core]
            m[f"wlin{li}"] = wlins[li][core]
        in_maps.append(m)

    res = run_bass_kernel_spmd(nc, in_maps, core_ids=list(range(NCORES)))
    out = np.zeros((B, OUTS[2]), np.float32)
    for core in range(NCORES):
        out += res.results[core]["out"]
    return out


# revision 26
# speedup vs baseline: 1.0930x; 1.0930x over previous
"""Trainium2 Bass kernel for nn_CubicModelLarge (3-layer cubic-feature MLP).

Tensor-parallel over the cubic min-index p (64 values, 8 per core; core c,
slot s -> p = 8s + c).  Monomial folding: each cubic monomial x_p x_q x_r
(p<=q<=r) is accumulated once into block p, contracting the triu pair basis

  u_(q,r) = (x_q + x_r)^2 / 2   (q < r, q-major tail order)
  sq_q    = x_q^2
  x_k     = x_k                 (carries the folded quadratic weights)

Block p only needs rows with q >= p, so chunk-level tail skipping cuts the
streamed GEMM columns ~2x vs the unfolded basis (slot-prefix packing keeps
the schedule SPMD-uniform; shorter-tail cores just carry zero weights).

Per core, per layer:
  H[b,(s,o)] = sum_rows F[row,b] * Wfold[row,(s,o)]    (fp16 GEMM, fp32 PSUM)
  y_c[b,o]   = lin[b,o] + sum_s xmac[b,s] * H[b,(s,o)]  (DVE mult+reduce)
  y          = AllReduce_c(y_c)

The batch is processed in thirds (384/384/256) with one fp16 AllReduce per
third, so each collective's latency hides behind the other thirds' compute.
u rows are built by a selection-SUM matmul on the PE (two 1s per column)
followed by a Square activation on the Scalar engine (PSUM->SBUF fp16).
A tiny warm-up AllReduce issues first to absorb collective-init latency.
Final layer partials are summed on the host.
"""

import numpy as np

D = 64
B = 1024
NCORES = 8
NSLOT = D // NCORES          # 8 slots (i-values) per core
OUTS = (64, 64, 10)
NUC = 16                     # u chunks (2016 rows + pad)
NCHUNK = 17                  # + 1 [sq; x] chunk
INV_SQRT2 = 0.7071067811865476

THIRDS = [(0, 512), (512, 1024)]
NT = len(THIRDS)

# slot s covers p in [8s, 8s+8); its tail starts at u-row off(8s)
_OFF = [q * (127 - q) // 2 for q in range(D)]
SLOT_START = [_OFF[8 * s] // 128 for s in range(NSLOT)]      # [0,3,6,9,11,13,14,15]
NSLOTS_AT = [sum(1 for s in range(NSLOT) if SLOT_START[s] <= c) for c in range(NUC)] + [NSLOT]
CHUNK_ORDER = [15, 16, 14, 13, 12, 11, 10, 9, 8, 7, 6, 5, 4, 3, 2, 1, 0]

_CACHE = {}


# ---------------------------------------------------------------- host prep --

def _pair_rows():
    """u-row index map: rows 0..2015 are pairs (q<r) q-major."""
    Q = np.zeros(2016, np.int64)
    R = np.zeros(2016, np.int64)
    for q in range(D):
        o = _OFF[q]
        n = 63 - q
        Q[o:o + n] = q
        R[o:o + n] = np.arange(q + 1, D)
    return Q, R


def _fold_blocks(W, out):
    """-> G [17*128, 64(p), out] folded coefficients per block p."""
    W_sq = W[:, D:D + 2080]
    W_cu = W[:, D + 2080:].reshape(out, D, 2080)
    iu, ju = np.triu_indices(D)

    # T3[p, q, r, out]: sum of W_cu[o, i, (j,k)] over placements, sorted triple
    T3 = np.zeros((D, D, D, out), np.float32)
    I = np.repeat(np.arange(D), 2080)
    J = np.tile(iu, D)
    K = np.tile(ju, D)
    S = np.sort(np.stack([I, J, K]), axis=0)
    V = W_cu.transpose(1, 2, 0).reshape(-1, out)
    np.add.at(T3, (S[0], S[1], S[2]), V)

    Q, R = _pair_rows()
    G = np.zeros((NCHUNK * 128, D, out), np.float32)
    # u-rows: G[row(q,r), p] = T3[p, q, r]  (zero when q < p by construction)
    G[:2016] = T3[:, Q, R, :].transpose(1, 0, 2)
    # sq-rows: diag cubic minus u-substitution corrections
    rowsum = T3.sum(axis=2)                     # [p, q, out] : sum_r T3[p,q,r]
    colsum = T3.sum(axis=1)                     # [p, r, out] : sum_q T3[p,q,r]
    diag = T3[:, np.arange(D), np.arange(D), :]  # [p, q, out]
    sqco = diag - 0.5 * (rowsum + colsum - 2 * diag)
    G[2048:2048 + D] = sqco.transpose(1, 0, 2)
    # x-rows: folded quadratic, pairs with min = p
    tmap = np.zeros((D, D), np.int64)
    tmap[iu, ju] = np.arange(2080)
    tmap[ju, iu] = tmap[iu, ju]
    Wsym = W_sq[:, tmap]                        # [out, p, k]
    mask = (np.arange(D)[None, :] >= np.arange(D)[:, None]).astype(np.float32)
    G[2112:2112 + D] = (Wsym * mask[None]).transpose(2, 1, 0)
    return G


def _prep_layer(W, b, out):
    """-> (wcub [NCORES](17*128, NSLOT*out) fp16, wlin [NCORES](65, out) fp16)"""
    G = _fold_blocks(W, out)
    wcubs, wlins = [], []
    for core in range(NCORES):
        wcub = np.zeros((NCHUNK * 128, NSLOT * out), np.float32)
        for s in range(NSLOT):
            wcub[:, s * out:(s + 1) * out] = G[:, 8 * s + core, :]
        wcubs.append(np.ascontiguousarray(wcub.astype(np.float16)))
        wl = np.zeros((65, out), np.float32)
        if core == 0:
            wl[:D] = W[:, :D].T
            wl[D] = b
        wlins.append(wl.astype(np.float16))
    return wcubs, wlins


def _sel_consts():
    """Selection-SUM matrices (64, 17*128), fp16.

    chunk c<16, col p: +1 at rows Q[128c+p], R[128c+p] (zero cols past 2016).
    chunk 16: col a (a<64): +1 at row a (builds x_a, squared to x_a^2).
    """
    Q, R = _pair_rows()
    sel = np.zeros((D, NCHUNK * 128), np.float16)
    for rho in range(2016):
        sel[Q[rho], rho] += 1.0
        sel[R[rho], rho] += 1.0
    for a in range(D):
        sel[a, NUC * 128 + a] += 1.0
    return sel


# ------------------------------------------------------------------ builder --

def _build_module():
    import concourse.bacc as bacc
    import concourse.mybir as mybir
    import concourse.tile as tile

    F32 = mybir.dt.float32
    F16 = mybir.dt.float16
    MULT = mybir.AluOpType.mult
    ADD = mybir.AluOpType.add
    SQUARE = mybir.ActivationFunctionType.Square
    AXIS_X = mybir.AxisListType.X

    nc = bacc.Bacc("TRN2", target_bir_lowering=False, num_devices=NCORES, debug=False)

    x_in = nc.dram_tensor("x", [B, D], F32, kind="ExternalInput")
    wcub_in = [
        nc.dram_tensor(f"wcub{li}", [NCHUNK * 128, NSLOT * OUTS[li]], F16, kind="ExternalInput")
        for li in range(3)
    ]
    wlin_in = [
        nc.dram_tensor(f"wlin{li}", [65, OUTS[li]], F16, kind="ExternalInput")
        for li in range(3)
    ]
    colsel_in = nc.dram_tensor("colsel", [D, NSLOT], F16, kind="ExternalInput")
    out_ext = nc.dram_tensor("out", [B, OUTS[2]], F32, kind="ExternalOutput")

    sel_c = nc.inline_tensor(_sel_consts(), name="selc")
    ident_c = nc.inline_tensor(np.eye(128, dtype=np.float32), name="identc")
    ident16_c = nc.inline_tensor(np.eye(128, dtype=np.float16), name="ident16c")

    with tile.TileContext(nc) as tc:
        with (
            tc.tile_pool(name="wpool", bufs=2) as wpool,
            tc.tile_pool(name="spool", bufs=1) as spool,
            tc.tile_pool(name="xpool", bufs=2) as xpool,
            tc.tile_pool(name="qpool", bufs=1) as qpool,
            tc.tile_pool(name="ypool", bufs=2) as ypool,
            tc.tile_pool(name="hpool", bufs=3) as hpool,
            tc.tile_pool(name="ps_rep", bufs=2, space="PSUM") as ps_rep,
            tc.tile_pool(name="ps_h", bufs=3, space="PSUM") as ps_h,
            tc.tile_pool(name="ps_small", bufs=3, space="PSUM") as ps_small,
            tc.tile_pool(name="dpool", bufs=2, space="DRAM") as dpool,
        ):
            # ---- warm-up collective: absorb ncfw init + cross-core skew
            warm_src = dpool.tile([128, 4], F16, tag="warm_src")
            warm_dst = dpool.tile([128, 4], F16, tag="warm_dst")
            warm_sb = spool.tile([128, 4], F16, tag="warm_sb")
            nc.vector.memset(warm_sb[:], 0.0)
            nc.sync.dma_start(warm_src[:], warm_sb[:])
            nc.gpsimd.collective_compute(
                "AllReduce",
                ADD,
                replica_groups=[list(range(NCORES))],
                ins=[warm_src.opt()],
                outs=[warm_dst.opt()],
            )

            sel_sb = spool.tile([D, NCHUNK * 128], F16, tag="sel")
            nc.scalar.dma_start(sel_sb[:], sel_c.ap())
            ident_sb = spool.tile([128, 128], F32, tag="ident")
            nc.scalar.dma_start(ident_sb[:], ident_c.ap())
            ident16_sb = spool.tile([128, 128], F16, tag="ident16")
            nc.scalar.dma_start(ident16_sb[:], ident16_c.ap())
            colsel_sb = spool.tile([D, NSLOT], F16, tag="colsel")
            nc.scalar.dma_start(colsel_sb[:], colsel_in.ap())

            # per-layer weight tiles; only the active slot-prefix per chunk.
            # weight DMAs ride the vector queue so they never head-of-line
            # block the latency-critical bounce/x DMAs on the sync queue.
            weights = []
            for li in range(3):
                out_l = OUTS[li]
                M = NSLOT * out_l
                wcub_sb = wpool.tile([128, NCHUNK, M], F16, tag="wcub")
                for c in range(NCHUNK):
                    w = out_l * NSLOTS_AT[c]
                    nc.scalar.dma_start(
                        wcub_sb[:, c, 0:w],
                        wcub_in[li].ap()[c * 128:(c + 1) * 128, 0:w],
                    )
                wlin_sb = wpool.tile([65, out_l], F16, tag="wlin")
                nc.scalar.dma_start(wlin_sb[:], wlin_in[li].ap())
                weights.append((wcub_sb, wlin_sb))

            # x tiles for layer 0 (fp32 straight from the input)
            x_parts = []
            for t, (b0, b1) in enumerate(THIRDS):
                nbt = (b1 - b0) // 128
                xs = xpool.tile([128, nbt, D], F32, tag=f"x{t}")
                nc.sync.dma_start(
                    xs[:],
                    x_in.ap()[b0:b1, :].rearrange("(bc p) f -> p bc f", p=128),
                )
                x_parts.append(xs)

            for li in range(3):
                out_l = OUTS[li]
                M = NSLOT * out_l
                last = li == 2
                wcub_sb, wlin_sb = weights[li]
                next_x = [None] * NT

                for t, (b0, b1) in enumerate(THIRDS):
                    TB = b1 - b0
                    nbt = TB // 128
                    x_sb = x_parts[t]
                    idw = ident_sb if li == 0 else ident16_sb

                    # -- phase A: xT via PE transposes + cast
                    xT_sb = xpool.tile([65, TB], F16, tag=f"xT{t}")
                    for bc in range(nbt):
                        xTp = ps_small.tile([D, 128], F32 if li == 0 else F16, tag="small")
                        nc.tensor.transpose(xTp[:], x_sb[:, bc, :], idw[:])
                        nc.scalar.copy(xT_sb[0:D, bc * 128:(bc + 1) * 128], xTp[:])
                    nc.vector.memset(xT_sb[D:65, :], 1.0)

                    # -- phase B: u chunks (sel-sum matmul + Square); chunk 16 = [sq; x]
                    xsq = []
                    for c in range(NUC):
                        rep = ps_rep.tile([128, TB], F32, tag="rep")
                        nc.tensor.matmul(
                            rep[:], sel_sb[:, c * 128:(c + 1) * 128],
                            xT_sb[0:D, :], start=True, stop=True,
                        )
                        xq = qpool.tile([128, TB], F16, tag=f"xsq{c}t{t}")
                        nc.scalar.activation(xq[:], rep[:], SQUARE, scale=INV_SQRT2)
                        xsq.append(xq)
                    rep16 = ps_rep.tile([128, TB], F32, tag="rep")
                    nc.tensor.matmul(
                        rep16[0:D, :], sel_sb[:, NUC * 128:NUC * 128 + D],
                        xT_sb[0:D, :], start=True, stop=True,
                    )
                    xq16 = qpool.tile([128, TB], F16, tag=f"xsq16t{t}")
                    nc.scalar.activation(xq16[0:D, :], rep16[0:D, :], SQUARE, scale=1.0)
                    nc.vector.tensor_copy(xq16[D:128, :], xT_sb[0:D, :])
                    xsq.append(xq16)

                    # -- phase C
                    y_sb = ypool.tile([128, nbt, out_l], F16, tag=f"y{t}")
                    for bc in range(nbt):
                        bs = slice(bc * 128, (bc + 1) * 128)
                        h_ps = ps_h.tile([128, M], F32, tag="h")
                        for j, c in enumerate(CHUNK_ORDER):
                            w = out_l * NSLOTS_AT[c]
                            nc.tensor.matmul(
                                h_ps[:, 0:w], xsq[c][:, bs], wcub_sb[:, c, 0:w],
                                start=(j == 0), stop=(j == NCHUNK - 1),
                            )

                        lin_ps = ps_small.tile([128, out_l], F32, tag="small")
                        nc.tensor.matmul(lin_ps[:], xT_sb[0:65, bs], wlin_sb[:], start=True, stop=True)
                        xmac_ps = ps_small.tile([128, NSLOT], F32, tag="small")
                        nc.tensor.matmul(xmac_ps[:], xT_sb[0:D, bs], colsel_sb[:], start=True, stop=True)
                        xmac_sb = ypool.tile([128, NSLOT], F32, tag="xmac")
                        nc.scalar.copy(xmac_sb[:], xmac_ps[:])

                        # tmp[:, :M] = h * xmac (broadcast over o); tmp[:, M:] = lin
                        tmp_sb = hpool.tile([128, M + out_l], F32, tag="tmp")
                        xmac_b = (
                            xmac_sb[:].unsqueeze(2).to_broadcast([128, NSLOT, out_l])
                        )
                        nc.vector.tensor_tensor(
                            tmp_sb[:, 0:M].rearrange("p (i o) -> p i o", i=NSLOT),
                            h_ps[:].rearrange("p (i o) -> p i o", i=NSLOT),
                            xmac_b,
                            op=MULT,
                        )
                        nc.scalar.copy(tmp_sb[:, M:], lin_ps[:])
                        with nc.allow_low_precision(reason="y partials are fp16 for the collective"):
                            nc.vector.tensor_reduce(
                                y_sb[:, bc, :],
                                tmp_sb[:].rearrange("p (i o) -> p o i", i=NSLOT + 1),
                                axis=AXIS_X, op=ADD,
                            )

                    if not last:
                        # -- phase D: AllReduce this third (fp16 payload)
                        y_bounce = dpool.tile([TB, out_l], F16, tag=f"ybounce{t}")
                        y_red = dpool.tile([TB, out_l], F16, tag=f"yred{t}")
                        nc.sync.dma_start(
                            y_bounce[:].rearrange("(bc p) o -> p bc o", p=128), y_sb[:]
                        )
                        nc.gpsimd.collective_compute(
                            "AllReduce",
                            ADD,
                            replica_groups=[list(range(NCORES))],
                            ins=[y_bounce.opt()],
                            outs=[y_red.opt()],
                        )
                        xs = xpool.tile([128, nbt, D], F16, tag=f"xn{t}")
                        nc.sync.dma_start(
                            xs[:], y_red[:].rearrange("(bc p) f -> p bc f", p=128)
                        )
                        next_x[t] = xs
                    else:
                        y32_sb = ypool.tile([128, nbt, out_l], F32, tag=f"y32{t}")
                        nc.scalar.copy(y32_sb[:], y_sb[:])
                        nc.sync.dma_start(
                            out_ext.ap()[b0:b1, :]
                            .rearrange("(bc p) o -> p bc o", p=128),
                            y32_sb[:],
                        )

                if not last:
                    x_parts = next_x

    nc.compile()
    return nc


# ------------------------------------------------------------------- runner --

def kernel(x, W0, b0, W1, b1, W2, b2):
    from concourse.bass_utils import run_bass_kernel_spmd

    if "nc" not in _CACHE:
        _CACHE["nc"] = _build_module()
    nc = _CACHE["nc"]

    x = np.ascontiguousarray(np.asarray(x, np.float32))
    Ws = [np.asarray(W, np.float32) for W in (W0, W1, W2)]
    bs = [np.asarray(b_, np.float32) for b_ in (b0, b1, b2)]

    wcubs, wlins = {}, {}
    for li in range(3):
        wcubs[li], wlins[li] = _prep_layer(Ws[li], bs[li], OUTS[li])

    in_maps = []
    for core in range(NCORES):
        colsel = np.zeros((D, NSLOT), np.float16)
        for s in range(NSLOT):
            colsel[8 * s + core, s] = 1.0
        m = {"x": x, "colsel": colsel}
        for li in range(3):
            m[f"wcub{li}"] = wcubs[li][core]
            m[f"wlin{li}"] = wlins[li][core]
        in_maps.append(m)

    res = run_bass_kernel_spmd(nc, in_maps, core_ids=list(range(NCORES)))
    out = np.zeros((B, OUTS[2]), np.float32)
    for core in range(NCORES):
        out += res.results[core]["out"]
    return out
